# revision 39
# baseline (speedup 1.0000x reference)
"""Windowed (patch) attention kernel for 8 Trainium2 NeuronCores.

Problem: serialized point-cloud attention.
  qkv = feat @ Wqkv + bqkv ; qkv = qkv[order] -> windows of 256 rows
  per-window, per-head softmax attention ; out = attn_out[inverse] @ Wproj + bproj

Distribution strategy (per sharding hint): the permutation `order` is applied
host-side while sharding, so each core receives its 32 windows' rows already
gathered and channel-major (transposed).  All FLOPs (QKV proj, attention,
output proj) run on-device in bf16 with f32 PSUM accumulation.  `inverse`
scatter + bias adds are applied host-side (exact; row permutation commutes
with the row-wise projection, softmax is shift-invariant so the k-bias
cancels and the v-bias contributes bv @ Wproj to every row).
"""

import numpy as np
import ml_dtypes

import concourse.mybir as mybir
from concourse import bacc
from concourse.tile import TileContext
from concourse.masks import make_identity
from concourse.bass_utils import run_bass_kernel_spmd

N = 65536
C = 512
H = 8
KW = 256          # window size
SCALE = 0.125
NCORES = 8
ROWS = N // NCORES        # 8192 rows per core
NWIN = ROWS // KW         # 32 windows per core
D = C // H                # 64 head dim

BF16 = mybir.dt.bfloat16
F32 = mybir.dt.float32
FP8 = mybir.dt.float8e4


def build_nc():
    nc = bacc.Bacc("TRN2", target_bir_lowering=False, debug=False, num_devices=NCORES)

    xt = nc.declare_dram_parameter("xt", [C, ROWS], BF16, isOutput=False)
    wqkv = nc.declare_dram_parameter("wqkv", [C, 3 * C], BF16, isOutput=False)
    wproj = nc.declare_dram_parameter("wproj", [C, C], BF16, isOutput=False)
    out = nc.declare_dram_parameter("out", [ROWS, C], F32, isOutput=True)

    with TileContext(nc, pool_alloc_mode="queue") as tc:
        with (
            tc.tile_pool(name="const", bufs=1) as const,
            tc.tile_pool(name="sb", bufs=4) as sb,
            tc.tile_pool(name="outp", bufs=3) as outp,
            tc.tile_pool(name="eh", bufs=3) as eh,
            tc.tile_pool(name="pqs", bufs=4, space="PSUM") as pqs,
            tc.tile_pool(name="pot", bufs=2, space="PSUM") as pot,
            tc.tile_pool(name="pvf", bufs=2, space="PSUM") as pvf,
        ):
            # --- persistent tiles: weights + identity -----------------------
            wq_sb = const.tile([128, 4, 3 * C], BF16)
            wp_sb = const.tile([128, 4, C], BF16)
            # split weight loads so the first matmul group isn't gated on the
            # whole 3 MB weight transfer
            wq_r = wqkv.rearrange("(j p) c -> p j c", p=128)
            for mc in range(6):
                nc.sync.dma_start(
                    out=wq_sb[:, :, mc * 256:(mc + 1) * 256],
                    in_=wq_r[:, :, mc * 256:(mc + 1) * 256],
                )
            nc.sync.dma_start(out=wp_sb, in_=wproj.rearrange("(j p) c -> p j c", p=128))

            # warm-up: keep the PE busy (and HAM un-throttled) while the
            # first weight/xt DMAs are in flight; results are discarded
            junk = const.tile([128, C], BF16)
            nc.vector.memset(junk, 0.0)
            ident = const.tile([128, 128], BF16)
            make_identity(nc, ident)
            wup = pqs.tile([128, C], F32, tag="qs", name="wup")
            for _ in range(24):
                nc.tensor.matmul(wup, lhsT=junk[:, 0:128], rhs=junk, start=True, stop=True)
            GW = 2 * KW  # 2 windows per group: N=512 matmuls for qkv

            def xt_load(wg):
                """Prefetch a group's x^T slab (issued one group early)."""
                g0 = wg * GW
                xt_g = sb.tile([128, 4, GW], BF16, name="xt_g")
                nc.gpsimd.dma_start(
                    out=xt_g,
                    in_=xt.rearrange("(j p) r -> p j r", p=128)[:, :, g0:g0 + GW],
                )
                return xt_g

            def qkv_emit(wg, xt_g):
                """Emit qkv projection for group wg; yields after each PE matmul.
                First yield delivers (qkT, v_sb) tile handles."""
                g0 = wg * GW
                # qk^T: q,k channels (1024) chunked by 128 -> [128, 8, 512]
                qkT = sb.tile([128, 8, GW], BF16, name="qkT")
                # v natural layout with ones column: [128rows, rc, head, 65]
                v_sb = sb.tile([128, 4, H, D + 1], BF16, name="v_sb")
                nc.vector.memset(v_sb[:, :, :, D:D + 1], 1.0)
                yield (qkT, v_sb)
                for m in range(8):
                    acc = pqs.tile([128, GW], F32, tag="qs", name="acc_qk")
                    for j in range(4):
                        nc.tensor.matmul(
                            acc,
                            lhsT=wq_sb[:, j, m * 128:(m + 1) * 128],
                            rhs=xt_g[:, j, :],
                            start=(j == 0),
                            stop=(j == 3),
                        )
                        yield None
                    nc.vector.tensor_copy(qkT[:, m, :], acc)
                for rc in range(4):
                    acc = pvf.tile([128, C], F32, tag="vf", name="acc_v")
                    for j in range(4):
                        nc.tensor.matmul(
                            acc,
                            lhsT=xt_g[:, j, rc * 128:(rc + 1) * 128],
                            rhs=wq_sb[:, j, 2 * C:3 * C],
                            start=(j == 0),
                            stop=(j == 3),
                        )
                        yield None
                    # [128, 512] viewed as (H, D) -> strided into (H, 65) slots
                    nc.vector.tensor_copy(
                        v_sb[:, rc, :, 0:D],
                        acc.rearrange("p (h d) -> p h d", h=H),
                    )

            pending_proj = []

            def proj_emit():
                """Emit the oldest pending projection (deferred one window so
                the attnT DMA-transpose has a window of slack)."""
                attnT, r0 = pending_proj.pop(0)
                for rc in range(2):
                    acc = pvf.tile([128, C], F32, tag="vf", name="acc_p")
                    for cc in range(4):
                        nc.tensor.matmul(
                            acc,
                            lhsT=attnT[:, cc, rc * 128:(rc + 1) * 128],
                            rhs=wp_sb[:, cc, :],
                            start=(cc == 0),
                            stop=(cc == 3),
                        )
                        yield None
                    o_sb = outp.tile([128, C], F32, name="o_sb")
                    nc.vector.tensor_copy(o_sb, acc)
                    nc.sync.dma_start(
                        out=out[r0 + rc * 128:r0 + (rc + 1) * 128, :], in_=o_sb
                    )

            def att_emit(wg, qkT, v_sb, flush=True, last=False):
                """Emit attention + projection for group wg; yields after each
                PE matmul so it can interleave with the next group's qkv."""
                g0 = wg * GW
                for wi in range(2):
                    r0 = g0 + wi * KW
                    qoff = wi * KW
                    attn = sb.tile([128, 2, C], BF16, name="attn", bufs=4)
                    for hp in range(4):
                        scs = []
                        exps = []
                        for hh in range(2):
                            scs.append(pqs.tile([128, 2, KW], F32, tag="qs", name=f"sc{hh}"))
                            exps.append(eh.tile([128, 2, KW], BF16, tag="expT", name=f"expT{hh}", bufs=4))
                        for kc in range(2):
                            for hh in range(2):
                                poff = hh * 64
                                nc.tensor.matmul(
                                    scs[hh][:, kc, :],
                                    lhsT=qkT[poff:poff + 64, 4 + hp,
                                             qoff + kc * 128:qoff + (kc + 1) * 128],
                                    rhs=qkT[poff:poff + 64, hp, qoff:qoff + KW],
                                    start=True,
                                    stop=True,
                                    tile_position=(poff, 0),
                                )
                                yield None
                        for hh in range(2):
                            # expT[k,q] = exp(scale*scoresT); no max-sub
                            # (|scores*scale| bounded ~8 for these inputs)
                            nc.scalar.activation(
                                exps[hh], scs[hh],
                                mybir.ActivationFunctionType.Exp,
                                scale=SCALE,
                            )
                        # both heads' attn@V into one PSUM bank: (qc, hh, 65)
                        ov = pot.tile([128, 2, 2, D + 1], F32, tag="ot", name="ov")
                        for hh in range(2):
                            h = 2 * hp + hh
                            for qc in range(2):
                                for kc in range(2):
                                    # 'v' asks the driver for a long matmul
                                    # first so this one's LDWEIGHTS hides
                                    yield 'v'
                                    nc.tensor.matmul(
                                        ov[:, qc, hh, :],
                                        lhsT=exps[hh][:, kc, qc * 128:(qc + 1) * 128],
                                        rhs=v_sb[:, wi * 2 + kc, h, :],
                                        start=(kc == 0),
                                        stop=(kc == 1),
                                    )
                        # normalize by the ones-column result (both heads at once)
                        rcp = eh.tile([128, 2, 2, 1], F32, tag="rcp", name="rcp")
                        nc.vector.reciprocal(rcp, ov[:, :, :, D:D + 1])
                        nc.vector.tensor_mul(
                            attn[:, :, 2 * hp * D:(2 * hp + 2) * D].rearrange(
                                "p q (e d) -> p q e d", e=2
                            ),
                            ov[:, :, :, 0:D],
                            rcp.broadcast_to([128, 2, 2, D]),
                        )

                    # transpose attention output for the projection:
                    # DMA xbar transpose (bf16): [row, ch] -> [ch, row]
                    attnT = sb.tile([128, 4, KW], BF16, name="attnT", bufs=6)
                    if last and wi == 1:
                        # tail: PE transposes avoid the DMA-transpose latency
                        # when there is no filler left to hide it
                        for qc in range(2):
                            tp = pot.tile([128, 4, 128], BF16, tag="ot",
                                          name=f"tp{qc}")
                            for cc in range(4):
                                nc.tensor.transpose(
                                    tp[:, cc, :],
                                    attn[:, qc, cc * 128:(cc + 1) * 128],
                                    ident,
                                )
                                yield None
                            nc.vector.tensor_copy(
                                attnT[:, :, qc * 128:(qc + 1) * 128], tp
                            )
                    else:
                        for qc in range(2):
                            nc.sync.dma_start_transpose(
                                out=attnT[:, :, qc * 128:(qc + 1) * 128],
                                in_=attn[:, qc, :],
                            )
                    pending_proj.append((attnT, r0))
                    if flush and len(pending_proj) > 2:
                        yield from proj_emit()

            def drain(g):
                for _ in g:
                    pass

            # software pipeline: qkv(wg) emission interleaves with the
            # attention of group wg-1, so short attention matmuls' LDWEIGHTS
            # hide under long N=512 qkv matmuls.
            xt_cur = xt_load(0)
            xt_nxt = xt_load(1)
            qg = qkv_emit(0, xt_cur)
            tiles = next(qg)
            drain(qg)
            for wg in range(1, NWIN // 2):
                xt_cur = xt_nxt
                if wg + 1 < NWIN // 2:
                    xt_nxt = xt_load(wg + 1)
                qg = qkv_emit(wg, xt_cur)
                new_tiles = next(qg)
                ag = att_emit(wg - 1, *tiles)
                qkv_live = True
                att_live = True
                while qkv_live or att_live:
                    tag = None
                    if att_live:
                        tag = next(ag, StopIteration)
                        att_live = tag is not StopIteration
                    if qkv_live and (tag == 'v' or not att_live):
                        qkv_live = next(qg, StopIteration) is not StopIteration
                tiles = new_tiles
            # final group: use the deferred projections as 'v' filler
            ag = att_emit(NWIN // 2 - 1, *tiles, flush=False, last=True)
            pg = None
            while True:
                tag = next(ag, StopIteration)
                if tag is StopIteration:
                    break
                if tag == 'v':
                    if pg is None and pending_proj:
                        pg = proj_emit()
                    if pg is not None and next(pg, StopIteration) is StopIteration:
                        pg = None
            if pg is not None:
                drain(pg)
            while pending_proj:
                drain(proj_emit())
    nc.finalize()
    return nc


_NC_CACHE = None


def _get_nc():
    global _NC_CACHE
    if _NC_CACHE is None:
        _NC_CACHE = build_nc()
    return _NC_CACHE


def _prep_in_maps(feat, order, Wqkv, Wproj):
    xs = np.asarray(feat, dtype=np.float32)[np.asarray(order)]
    wq = np.asarray(Wqkv, dtype=np.float32).astype(ml_dtypes.bfloat16)
    wp = np.asarray(Wproj, dtype=np.float32).astype(ml_dtypes.bfloat16)
    in_maps = []
    for m in range(NCORES):
        shard = xs[m * ROWS:(m + 1) * ROWS]
        xtb = np.ascontiguousarray(shard.T).astype(ml_dtypes.bfloat16)
        in_maps.append({"xt": xtb, "wqkv": wq, "wproj": wp})
    return in_maps


def kernel(feat, order, inverse, Wqkv, bqkv, Wproj, bproj, _trace=False):
    nc = _get_nc()
    in_maps = _prep_in_maps(feat, order, Wqkv, Wproj)
    res = run_bass_kernel_spmd(nc, in_maps, core_ids=list(range(NCORES)), trace=_trace)
    serial = np.concatenate([r["out"] for r in res.results], axis=0)
    final = serial[np.asarray(inverse)]
    # biases (host-side, exact): v-bias rides through softmax (rows sum to 1)
    # as + bv @ Wproj ; k-bias cancels in softmax ; q-bias is zero by spec.
    total_bias = (
        np.asarray(bqkv, dtype=np.float32)[2 * C:3 * C] @ np.asarray(Wproj, dtype=np.float32)
        + np.asarray(bproj, dtype=np.float32)
    )
    out = final + total_bias[None, :]
    if _trace:
        return out.astype(np.float32), res
    return out.astype(np.float32)


# revision 40
# speedup vs baseline: 1.0049x; 1.0049x over previous
"""Windowed (patch) attention kernel for 8 Trainium2 NeuronCores.

Problem: serialized point-cloud attention.
  qkv = feat @ Wqkv + bqkv ; qkv = qkv[order] -> windows of 256 rows
  per-window, per-head softmax attention ; out = attn_out[inverse] @ Wproj + bproj

Distribution strategy (per sharding hint): the permutation `order` is applied
host-side while sharding, so each core receives its 32 windows' rows already
gathered and channel-major (transposed).  All FLOPs (QKV proj, attention,
output proj) run on-device in bf16 with f32 PSUM accumulation.  `inverse`
scatter + bias adds are applied host-side (exact; row permutation commutes
with the row-wise projection, softmax is shift-invariant so the k-bias
cancels and the v-bias contributes bv @ Wproj to every row).
"""

import numpy as np
import ml_dtypes

import concourse.mybir as mybir
from concourse import bacc
from concourse.tile import TileContext
from concourse.masks import make_identity
from concourse.bass_utils import run_bass_kernel_spmd

N = 65536
C = 512
H = 8
KW = 256          # window size
SCALE = 0.125
NCORES = 8
ROWS = N // NCORES        # 8192 rows per core
NWIN = ROWS // KW         # 32 windows per core
D = C // H                # 64 head dim

BF16 = mybir.dt.bfloat16
F32 = mybir.dt.float32
FP8 = mybir.dt.float8e4


def build_nc():
    nc = bacc.Bacc("TRN2", target_bir_lowering=False, debug=False, num_devices=NCORES)

    xt = nc.declare_dram_parameter("xt", [C, ROWS], BF16, isOutput=False)
    wqkv = nc.declare_dram_parameter("wqkv", [C, 3 * C], BF16, isOutput=False)
    wproj = nc.declare_dram_parameter("wproj", [C, C], BF16, isOutput=False)
    out = nc.declare_dram_parameter("out", [ROWS, C], F32, isOutput=True)

    with TileContext(nc, pool_alloc_mode="queue") as tc:
        with (
            tc.tile_pool(name="const", bufs=1) as const,
            tc.tile_pool(name="sb", bufs=4) as sb,
            tc.tile_pool(name="outp", bufs=4) as outp,
            tc.tile_pool(name="eh", bufs=3) as eh,
            tc.tile_pool(name="pqs", bufs=4, space="PSUM") as pqs,
            tc.tile_pool(name="pot", bufs=2, space="PSUM") as pot,
            tc.tile_pool(name="pvf", bufs=2, space="PSUM") as pvf,
        ):
            # --- persistent tiles: weights + identity -----------------------
            wq_sb = const.tile([128, 4, 3 * C], BF16)
            wp_sb = const.tile([128, 4, C], BF16)
            # split weight loads so the first matmul group isn't gated on the
            # whole 3 MB weight transfer
            wq_r = wqkv.rearrange("(j p) c -> p j c", p=128)
            for mc in range(6):
                nc.sync.dma_start(
                    out=wq_sb[:, :, mc * 256:(mc + 1) * 256],
                    in_=wq_r[:, :, mc * 256:(mc + 1) * 256],
                )
            nc.sync.dma_start(out=wp_sb, in_=wproj.rearrange("(j p) c -> p j c", p=128))

            # warm-up: keep the PE busy (and HAM un-throttled) while the
            # first weight/xt DMAs are in flight; results are discarded
            junk = const.tile([128, C], BF16)
            nc.vector.memset(junk, 0.0)
            ident = const.tile([128, 128], BF16)
            make_identity(nc, ident)
            wup = pqs.tile([128, C], F32, tag="qs", name="wup")
            for _ in range(24):
                nc.tensor.matmul(wup, lhsT=junk[:, 0:128], rhs=junk, start=True, stop=True)
            GW = 2 * KW  # 2 windows per group: N=512 matmuls for qkv

            def xt_load(wg):
                """Prefetch a group's x^T slab (issued one group early)."""
                g0 = wg * GW
                xt_g = sb.tile([128, 4, GW], BF16, name="xt_g")
                nc.gpsimd.dma_start(
                    out=xt_g,
                    in_=xt.rearrange("(j p) r -> p j r", p=128)[:, :, g0:g0 + GW],
                )
                return xt_g

            def qkv_emit(wg, xt_g):
                """Emit qkv projection for group wg; yields after each PE matmul.
                First yield delivers (qkT, v_sb) tile handles."""
                g0 = wg * GW
                # qk^T: q,k channels (1024) chunked by 128 -> [128, 8, 512]
                qkT = sb.tile([128, 8, GW], BF16, name="qkT")
                # v natural layout with ones column: [128rows, rc, head, 65]
                v_sb = sb.tile([128, 4, H, D + 1], BF16, name="v_sb")
                nc.vector.memset(v_sb[:, :, :, D:D + 1], 1.0)
                yield (qkT, v_sb)
                for m in range(8):
                    acc = pqs.tile([128, GW], F32, tag="qs", name="acc_qk")
                    for j in range(4):
                        nc.tensor.matmul(
                            acc,
                            lhsT=wq_sb[:, j, m * 128:(m + 1) * 128],
                            rhs=xt_g[:, j, :],
                            start=(j == 0),
                            stop=(j == 3),
                        )
                        yield None
                    nc.vector.tensor_copy(qkT[:, m, :], acc)
                for rc in range(4):
                    acc = pvf.tile([128, C], F32, tag="vf", name="acc_v")
                    for j in range(4):
                        nc.tensor.matmul(
                            acc,
                            lhsT=xt_g[:, j, rc * 128:(rc + 1) * 128],
                            rhs=wq_sb[:, j, 2 * C:3 * C],
                            start=(j == 0),
                            stop=(j == 3),
                        )
                        yield None
                    # [128, 512] viewed as (H, D) -> strided into (H, 65) slots
                    nc.vector.tensor_copy(
                        v_sb[:, rc, :, 0:D],
                        acc.rearrange("p (h d) -> p h d", h=H),
                    )

            pending_proj = []

            def proj_emit():
                """Emit the oldest pending projection (deferred one window so
                the attnT DMA-transpose has a window of slack)."""
                attnT, r0 = pending_proj.pop(0)
                for rc in range(2):
                    acc = pvf.tile([128, C], F32, tag="vf", name="acc_p")
                    for cc in range(4):
                        nc.tensor.matmul(
                            acc,
                            lhsT=attnT[:, cc, rc * 128:(rc + 1) * 128],
                            rhs=wp_sb[:, cc, :],
                            start=(cc == 0),
                            stop=(cc == 3),
                        )
                        yield None
                    o_sb = outp.tile([128, C], F32, name="o_sb")
                    nc.vector.tensor_copy(o_sb, acc)
                    nc.sync.dma_start(
                        out=out[r0 + rc * 128:r0 + (rc + 1) * 128, :], in_=o_sb
                    )

            def att_emit(wg, qkT, v_sb, flush=True, last=False):
                """Emit attention + projection for group wg; yields after each
                PE matmul so it can interleave with the next group's qkv."""
                g0 = wg * GW
                for wi in range(2):
                    r0 = g0 + wi * KW
                    qoff = wi * KW
                    attn = sb.tile([128, 2, C], BF16, name="attn", bufs=6)
                    for hp in range(4):
                        scs = []
                        exps = []
                        for hh in range(2):
                            scs.append(pqs.tile([128, 2, KW], F32, tag="qs", name=f"sc{hh}"))
                            exps.append(eh.tile([128, 2, KW], BF16, tag="expT", name=f"expT{hh}", bufs=4))
                        for kc in range(2):
                            for hh in range(2):
                                poff = hh * 64
                                nc.tensor.matmul(
                                    scs[hh][:, kc, :],
                                    lhsT=qkT[poff:poff + 64, 4 + hp,
                                             qoff + kc * 128:qoff + (kc + 1) * 128],
                                    rhs=qkT[poff:poff + 64, hp, qoff:qoff + KW],
                                    start=True,
                                    stop=True,
                                    tile_position=(poff, 0),
                                )
                                yield None
                        for hh in range(2):
                            # expT[k,q] = exp(scale*scoresT); no max-sub
                            # (|scores*scale| bounded ~8 for these inputs)
                            nc.scalar.activation(
                                exps[hh], scs[hh],
                                mybir.ActivationFunctionType.Exp,
                                scale=SCALE,
                            )
                        # both heads' attn@V into one PSUM bank: (qc, hh, 65)
                        ov = pot.tile([128, 2, 2, D + 1], F32, tag="ot", name="ov")
                        for hh in range(2):
                            h = 2 * hp + hh
                            for qc in range(2):
                                for kc in range(2):
                                    # 'v' asks the driver for a long matmul
                                    # first so this one's LDWEIGHTS hides
                                    yield 'v'
                                    nc.tensor.matmul(
                                        ov[:, qc, hh, :],
                                        lhsT=exps[hh][:, kc, qc * 128:(qc + 1) * 128],
                                        rhs=v_sb[:, wi * 2 + kc, h, :],
                                        start=(kc == 0),
                                        stop=(kc == 1),
                                    )
                        # normalize by the ones-column result (both heads at once)
                        rcp = eh.tile([128, 2, 2, 1], F32, tag="rcp", name="rcp")
                        nc.vector.reciprocal(rcp, ov[:, :, :, D:D + 1])
                        nc.vector.tensor_mul(
                            attn[:, :, 2 * hp * D:(2 * hp + 2) * D].rearrange(
                                "p q (e d) -> p q e d", e=2
                            ),
                            ov[:, :, :, 0:D],
                            rcp.broadcast_to([128, 2, 2, D]),
                        )

                    # transpose attention output for the projection:
                    # DMA xbar transpose (bf16): [row, ch] -> [ch, row]
                    attnT = sb.tile([128, 4, KW], BF16, name="attnT", bufs=6)
                    if last and wi == 1:
                        # tail: PE transposes avoid the DMA-transpose latency
                        # when there is no filler left to hide it
                        for qc in range(2):
                            tp = pot.tile([128, 4, 128], BF16, tag="ot",
                                          name=f"tp{qc}")
                            for cc in range(4):
                                nc.tensor.transpose(
                                    tp[:, cc, :],
                                    attn[:, qc, cc * 128:(cc + 1) * 128],
                                    ident,
                                )
                                yield None
                            nc.vector.tensor_copy(
                                attnT[:, :, qc * 128:(qc + 1) * 128], tp
                            )
                    else:
                        for qc in range(2):
                            nc.sync.dma_start_transpose(
                                out=attnT[:, :, qc * 128:(qc + 1) * 128],
                                in_=attn[:, qc, :],
                            )
                    pending_proj.append((attnT, r0))
                    if flush and len(pending_proj) > 2:
                        yield from proj_emit()

            def drain(g):
                for _ in g:
                    pass

            # software pipeline: qkv(wg) emission interleaves with the
            # attention of group wg-1, so short attention matmuls' LDWEIGHTS
            # hide under long N=512 qkv matmuls.
            xt_cur = xt_load(0)
            xt_nxt = xt_load(1)
            qg = qkv_emit(0, xt_cur)
            tiles = next(qg)
            drain(qg)
            for wg in range(1, NWIN // 2):
                xt_cur = xt_nxt
                if wg + 1 < NWIN // 2:
                    xt_nxt = xt_load(wg + 1)
                qg = qkv_emit(wg, xt_cur)
                new_tiles = next(qg)
                ag = att_emit(wg - 1, *tiles)
                qkv_live = True
                att_live = True
                while qkv_live or att_live:
                    tag = None
                    if att_live:
                        tag = next(ag, StopIteration)
                        att_live = tag is not StopIteration
                    if qkv_live and (tag == 'v' or not att_live):
                        qkv_live = next(qg, StopIteration) is not StopIteration
                tiles = new_tiles
            # final group: use the deferred projections as 'v' filler
            ag = att_emit(NWIN // 2 - 1, *tiles, flush=False, last=True)
            pg = None
            while True:
                tag = next(ag, StopIteration)
                if tag is StopIteration:
                    break
                if tag == 'v':
                    if pg is None and pending_proj:
                        pg = proj_emit()
                    if pg is not None and next(pg, StopIteration) is StopIteration:
                        pg = None
            if pg is not None:
                drain(pg)
            while pending_proj:
                drain(proj_emit())
    nc.finalize()
    return nc


_NC_CACHE = None


def _get_nc():
    global _NC_CACHE
    if _NC_CACHE is None:
        _NC_CACHE = build_nc()
    return _NC_CACHE


def _prep_in_maps(feat, order, Wqkv, Wproj):
    xs = np.asarray(feat, dtype=np.float32)[np.asarray(order)]
    wq = np.asarray(Wqkv, dtype=np.float32).astype(ml_dtypes.bfloat16)
    wp = np.asarray(Wproj, dtype=np.float32).astype(ml_dtypes.bfloat16)
    in_maps = []
    for m in range(NCORES):
        shard = xs[m * ROWS:(m + 1) * ROWS]
        xtb = np.ascontiguousarray(shard.T).astype(ml_dtypes.bfloat16)
        in_maps.append({"xt": xtb, "wqkv": wq, "wproj": wp})
    return in_maps


def kernel(feat, order, inverse, Wqkv, bqkv, Wproj, bproj, _trace=False):
    nc = _get_nc()
    in_maps = _prep_in_maps(feat, order, Wqkv, Wproj)
    res = run_bass_kernel_spmd(nc, in_maps, core_ids=list(range(NCORES)), trace=_trace)
    serial = np.concatenate([r["out"] for r in res.results], axis=0)
    final = serial[np.asarray(inverse)]
    # biases (host-side, exact): v-bias rides through softmax (rows sum to 1)
    # as + bv @ Wproj ; k-bias cancels in softmax ; q-bias is zero by spec.
    total_bias = (
        np.asarray(bqkv, dtype=np.float32)[2 * C:3 * C] @ np.asarray(Wproj, dtype=np.float32)
        + np.asarray(bproj, dtype=np.float32)
    )
    out = final + total_bias[None, :]
    if _trace:
        return out.astype(np.float32), res
    return out.astype(np.float32)
